# revision 49
# baseline (speedup 1.0000x reference)
"""Trainium2 Bass kernel for nn_MoELayerStacks (moe_routing).

Full inputs in, full output out. Data-parallel over batch across 8 cores.

Math (per batch row b):
  gate = [x[:32], x[1536:1568]] @ router_w.T + router_b           # [8]
  idx  = argmax(gate)
  l1c  = x @ l1_w[e].T + l1_b[e]   for all e                      # [8, 16]
  l1x  = clip([square(l1c[:, :15])*255/256, l1c[:, :15]], 0, 1)   # [8, 30]
  l2x  = clip(l1x @ l2_w[e].T + l2_b[e], 0, 1)                    # [8, 32]
  out  = (l2x @ out_w[e].T + out_b[e] + l1c[:, 15])[idx]          # [1]

v2 design (vs the fp32r v1): x and the expert weights are cast to fp16 on
the host, halving HBM->SBUF traffic (the dominant cost) and keeping the PE
at 1 cyc/row. The 64 router features ride in a separate fp32 sidecar so the
argmax sees the exact fp32 gate dot products (identical numerics to v1).

Layouts: features on partitions, batch on the free dim for l1/l2. Stacked
l1 feature index r(e,o): l1x features (k = o*8+e) at r = k for k < 64 and
r = k+8 for k >= 64; the 8 l1x_out features at r = 64+e so a lane-aligned
DVE copy can drop them into rows 64..71 of the fp32 gate-stationary tile.

The gate, l3, and argmax-select run batch-major without any PE transposes:
per 128-column chunk j, one fp32 matmul with stationary xr_ext[:, j] (64 xr
rows + 8 l1x_out rows + a ones row for router_b) and moving wcomb [97, 16]
produces gate (cols 0..7) and l1x_out (cols 8..15, via an identity block);
two fp16 matmuls with stationary l2a/l2b chunks and moving w3e accumulate
the l3 contribution into cols 8..15. A short DVE chain (reduce_max, is_ge,
mult, reduce_sum) then emits the selected expert output per batch row.
"""

import os
from contextlib import ExitStack

import numpy as np

import concourse.bacc as bacc
import concourse.mybir as mybir
import concourse.tile as tile

N_CORES = 8
B, L1, L2, L3, E = 16384, 3072, 15, 32, 8
RF = 32  # router feats per perspective
HALF = L1 // 2
B_SH = B // N_CORES  # 2048 rows per core
KC = L1 // 128  # 24 contraction chunks
SQ_SCALE = float(np.sqrt(255.0 / 256.0))
MB = 256  # batch columns per block
NB = B_SH // MB  # 8 blocks
NSUB = MB // 128  # 2 128-col chunks per block

F32 = mybir.dt.float32
F16 = mybir.dt.float16
ALU = mybir.AluOpType


def _stack_row(k):
    """Stacked l1 partition for l1x feature k = o*8+e (l1x_out at 64..71)."""
    return k if k < 64 else k + 8


def build_nc():
    nc = bacc.Bacc(dynamic_dma_scratch_size=16)

    xp = nc.dram_tensor("xp", [128, NB * KC * MB], F16, kind="ExternalInput")
    xr = nc.dram_tensor("xr", [2 * RF, B_SH], F32, kind="ExternalInput")
    w1t = nc.dram_tensor("w1t", [128, KC * 128], F16, kind="ExternalInput")
    cw16 = nc.dram_tensor("cw16", [128, 544], F16, kind="ExternalInput")
    cw32 = nc.dram_tensor("cw32", [128, 24], F32, kind="ExternalInput")
    y = nc.dram_tensor("y", [128, NB * NSUB], F32, kind="ExternalOutput")

    with tile.TileContext(nc) as tc, ExitStack() as ctx:
        const = ctx.enter_context(tc.tile_pool(name="const", bufs=1))
        xpool = ctx.enter_context(tc.tile_pool(name="x", bufs=6))
        actp = ctx.enter_context(tc.tile_pool(name="act", bufs=2))
        smallp = ctx.enter_context(tc.tile_pool(name="small", bufs=2))
        ps_1 = ctx.enter_context(tc.tile_pool(name="ps1", bufs=3, space="PSUM"))
        ps_2a = ctx.enter_context(tc.tile_pool(name="ps2a", bufs=1, space="PSUM"))
        ps_2b = ctx.enter_context(tc.tile_pool(name="ps2b", bufs=1, space="PSUM"))
        ps_sel = ctx.enter_context(tc.tile_pool(name="psel", bufs=2, space="PSUM"))

        w1t_sb = const.tile([128, KC, 128], F16)
        w1t_v = w1t[:, :].rearrange("p (c f) -> p c f", f=128)
        c16_sb = const.tile([128, 544], F16)
        c32_sb = const.tile([128, 24], F32)
        w2_sb = c16_sb[:, 0:512]
        w3a_sb = c16_sb[:, 512:528]
        w3b_sb = c16_sb[:, 528:544]
        wc_sb = c32_sb[:, 0:16]
        bias_sb = c32_sb[:, 16:24]
        xre = const.tile([128, B_SH], F32)  # rows 0..63 xr, 64..71 l1x_out, 96 ones
        yfull = const.tile([128, NB * NSUB], F32)

        def emit_consts():
            # everything the l1 matmuls and tails need, all on the sync queue
            # so x pieces on the other queues are never stuck behind them
            nc.sync.dma_start(c16_sb[:], cw16[:, :])
            nc.sync.dma_start(c32_sb[:], cw32[:, :])
            nc.scalar.dma_start(xre[0 : 2 * RF, :], xr[:, :])
            nc.vector.memset(xre[64:128, :], 0.0)
            nc.vector.memset(xre[96:97, :], 1.0)

        st = {}

        def emit_load(b, cuts=None, engs=None):
            xt = xpool.tile([128, KC, MB], F16, tag="xt")
            off = b * KC * MB
            cuts = cuts or [0, 6, 12, 18, 24]
            engs = engs or ([nc.sync, nc.scalar] * 8)
            src = xp[:, off : off + KC * MB].rearrange("p (c m) -> p c m", m=MB)
            for i in range(len(cuts) - 1):
                engs[i].dma_start(
                    xt[:, cuts[i] : cuts[i + 1], :], src[:, cuts[i] : cuts[i + 1], :]
                )
            st[b] = {"xt": xt}

        def emit_burst(b):
            xt = st[b]["xt"]
            ps1 = ps_1.tile([128, MB], F32, tag="ps1")
            for c in range(KC):
                nc.tensor.matmul(
                    ps1[:],
                    w1t_sb[:, c, :],
                    xt[:, c, :],
                    start=(c == 0),
                    stop=(c == KC - 1),
                )
            st[b]["ps1"] = ps1

        def emit_tail(b, nsplit=1, use_act=False):
            ps1 = st[b]["ps1"]
            psel = ps_sel.tile([128, NSUB, 16], F32, tag="psel")
            ps2a = ps_2a.tile([128, MB], F32, tag="ps2a")
            ps2b = ps_2b.tile([128, MB], F32, tag="ps2b")
            sq = actp.tile([128, MB], F16, tag="sq")
            raw = actp.tile([128, MB], F16, tag="raw")
            l2a = actp.tile([128, MB], F16, tag="l2a")
            l2b = actp.tile([128, MB], F16, tag="l2b")
            mx = smallp.tile([128, NSUB], F32, tag="mx")
            eq = smallp.tile([128, NSUB, 8], F32, tag="eq")
            prod = smallp.tile([128, NSUB, 8], F32, tag="prod")
            w = MB // nsplit
            for s in range(nsplit):
                cs = slice(s * w, (s + 1) * w)
                m0 = b * MB + s * w
                # l1x_out (+ l1 bias + out_b) into the fp32 gate-stationary rows
                if use_act:
                    nc.scalar.activation(
                        xre[64:72, m0 : m0 + w],
                        ps1[64:72, cs],
                        mybir.ActivationFunctionType.Identity,
                        bias=bias_sb[64:72, 4:5],
                    )
                else:
                    nc.vector.tensor_scalar(
                        xre[64:72, m0 : m0 + w],
                        ps1[64:72, cs],
                        bias_sb[64:72, 4:5],
                        None,
                        op0=ALU.add,
                    )
                # sq = min(1, (s*(l1c+b1))^2), raw = min(1, max(0, l1c+b1)).
                # PSUM-reading first steps on DVE/Act (GpSimd can't touch
                # PSUM); SBUF-only follow-ups on GpSimd — three parallel
                # elementwise tracks.
                nc.vector.tensor_scalar(
                    sq[:, cs],
                    ps1[:, cs],
                    bias_sb[:, 1:2],
                    SQ_SCALE,
                    op0=ALU.add,
                    op1=ALU.mult,
                )
                nc.vector.tensor_tensor(sq[:, cs], sq[:, cs], sq[:, cs], op=ALU.mult)
                nc.vector.tensor_scalar_min(sq[:, cs], sq[:, cs], 1.0)
                if use_act:
                    nc.scalar.activation(
                        raw[:, cs],
                        ps1[:, cs],
                        mybir.ActivationFunctionType.Relu,
                        bias=bias_sb[:, 1:2],
                    )
                else:
                    nc.vector.tensor_scalar(
                        raw[:, cs],
                        ps1[:, cs],
                        bias_sb[:, 1:2],
                        0.0,
                        op0=ALU.add,
                        op1=ALU.max,
                    )
                nc.vector.tensor_scalar_min(raw[:, cs], raw[:, cs], 1.0)

                # l2: two expert groups (0-3, 4-7), sq+raw accumulated
                nc.tensor.matmul(
                    ps2a[:, cs], w2_sb[:, 0:128], sq[:, cs], start=True, stop=False
                )
                nc.tensor.matmul(
                    ps2a[:, cs], w2_sb[:, 128:256], raw[:, cs], start=False, stop=True
                )
                nc.tensor.matmul(
                    ps2b[:, cs], w2_sb[:, 256:384], sq[:, cs], start=True, stop=False
                )
                nc.tensor.matmul(
                    ps2b[:, cs], w2_sb[:, 384:512], raw[:, cs], start=False, stop=True
                )

                if use_act:
                    nc.scalar.activation(
                        l2a[:, cs],
                        ps2a[:, cs],
                        mybir.ActivationFunctionType.Relu,
                        bias=bias_sb[:, 2:3],
                    )
                else:
                    nc.vector.tensor_scalar(
                        l2a[:, cs],
                        ps2a[:, cs],
                        bias_sb[:, 2:3],
                        0.0,
                        op0=ALU.add,
                        op1=ALU.max,
                    )
                nc.vector.tensor_scalar_min(l2a[:, cs], l2a[:, cs], 1.0)
                nc.vector.tensor_scalar(
                    l2b[:, cs], ps2b[:, cs], bias_sb[:, 3:4], 0.0, op0=ALU.add, op1=ALU.max
                )
                nc.vector.tensor_scalar_min(l2b[:, cs], l2b[:, cs], 1.0)

                # batch-major gate + all_outputs: per 128-col chunk j, PSUM
                # [128, 16]: cols 0..7 gate (fp32, exact), 8..15 l1x_out + l3c
                j0, j1 = s * (NSUB // nsplit), (s + 1) * (NSUB // nsplit)
                for j in range(j0, j1):
                    c0 = b * MB + j * 128
                    nc.tensor.matmul(
                        psel[:, j, :],
                        xre[0:97, c0 : c0 + 128],
                        wc_sb[0:97, :],
                        start=True,
                        stop=False,
                    )
                    nc.tensor.matmul(
                        psel[:, j, :],
                        l2a[:, j * 128 : (j + 1) * 128],
                        w3a_sb[:],
                        start=False,
                        stop=False,
                        skip_group_check=True,
                    )
                    nc.tensor.matmul(
                        psel[:, j, :],
                        l2b[:, j * 128 : (j + 1) * 128],
                        w3b_sb[:],
                        start=False,
                        stop=True,
                        skip_group_check=True,
                    )

                # argmax-select, batch on partitions
                js = slice(j0, j1)
                nc.vector.reduce_max(
                    mx[:, js], psel[:, js, 0:8], axis=mybir.AxisListType.X
                )
                for j in range(j0, j1):
                    nc.vector.tensor_scalar(
                        eq[:, j, :],
                        psel[:, j, 0:8],
                        mx[:, j : j + 1],
                        None,
                        op0=ALU.is_ge,
                    )
                nc.vector.tensor_tensor(
                    prod[:, js], eq[:, js], psel[:, js, 8:16], op=ALU.mult
                )
                ycols = slice(b * NSUB + j0, b * NSUB + j1)
                nc.vector.reduce_sum(
                    yfull[:, ycols], prod[:, js], axis=mybir.AxisListType.X
                )
            del st[b]

        # skewed software pipeline: PE runs burst(b) before tail(b-1) so the
        # DVE tail chain of block b-1 has a full block of slack. Two DMA
        # queues with disjoint roles: scalar carries only block 0 (fine
        # pieces for a fast PE start) and the router sidecar, then stays
        # empty; sync carries weights then blocks 1..3 in consumption order.
        # A single queue spreads its descriptors over all 16 DMA engines, so
        # bandwidth does not suffer. Tail elementwise work stays on the
        # vector engine except the last two blocks, where the scalar queue
        # has drained and the Act engine takes a parallel share.
        nc.sync.dma_start(w1t_sb[:, :, :], w1t_v[:, :, :])
        emit_load(
            0,
            cuts=[0, 3, 6, 12, 18, 24],
            engs=[nc.scalar, nc.sync, nc.scalar, nc.sync, nc.scalar],
        )
        emit_consts()
        emit_burst(0)
        for b in range(1, NB):
            if b == NB - 1:
                emit_load(
                    b,
                    cuts=[0, 12, 24],
                    engs=[nc.sync, nc.scalar],
                )
            else:
                emit_load(
                    b,
                    cuts=[0, 24],
                    engs=[nc.sync] if b % 2 else [nc.scalar],
                )
            emit_burst(b)
            emit_tail(b - 1, use_act=(b - 1 >= NB - 2))
        emit_tail(NB - 1, nsplit=2, use_act=True)
        nc.sync.dma_start(y[:, :], yfull[:])

    nc.finalize()
    return nc


def prep_weights(router_w, router_b, l1_w, l1_b, l2_w, l2_b, out_w, out_b):
    """Host-side packing of the (tiny) weights into the kernel's layouts."""
    f4, f2 = np.float32, np.float16
    # stacked l1 rows: l1x k=o*8+e -> r(k); l1x_out e -> 64+e
    w1_stacked = np.zeros((128, L1), f4)
    b1col = np.zeros(128, f4)
    for o in range(L2):
        for e in range(E):
            r = _stack_row(o * 8 + e)
            w1_stacked[r] = l1_w[e, o, :]
            b1col[r] = l1_b[e, o]
    for e in range(E):
        w1_stacked[64 + e] = l1_w[e, L2, :]
        b1col[64 + e] = l1_b[e, L2]
    w1t_kf = np.ascontiguousarray(w1_stacked.T).astype(f2)  # [L1, 128]
    # swizzle to [p, c, f] so the on-chip load is contiguous per partition
    w1t = np.ascontiguousarray(
        np.transpose(w1t_kf.reshape(KC, 128, 128), (1, 0, 2))
    ).reshape(128, KC * 128)
    # l2 weights: rows r(e,o), packed [sqA | rawA | sqB | rawB]
    w2p = np.zeros((128, 512), f4)
    for e in range(E):
        base = 0 if e < 4 else 256
        c0 = (e % 4) * 32
        wt = l2_w[e].T  # [30, 32]; rows 0..14 sq features, 15..29 raw
        rows = np.array([_stack_row(o * 8 + e) for o in range(L2)])
        w2p[rows, base + c0 : base + c0 + 32] = wt[0:L2]
        w2p[rows, base + 128 + c0 : base + 128 + c0 + 32] = wt[L2 : 2 * L2]
    w2p = w2p.astype(f2)
    # l3 (batch-major): w3p[:, g*16 + 8 + e] over the 32-feature band of e
    w3p = np.zeros((128, 32), f4)
    for e in range(E):
        g = e // 4
        w3p[(e % 4) * 32 : (e % 4) * 32 + 32, g * 16 + 8 + e] = out_w[e, 0, :]
    w3p = w3p.astype(f2)
    # wcomb: rows 0..63 router_w.T -> gate cols; rows 64..71 identity -> l1x_out
    # passthrough; row 96 (ones row in xre) carries router_b
    wcp = np.zeros((128, 16), f4)
    wcp[0 : 2 * RF, 0:8] = router_w.T
    for e in range(E):
        wcp[64 + e, 8 + e] = 1.0
    wcp[96, 0:8] = router_b
    biasp = np.zeros((128, 8), f4)
    biasp[:, 1] = b1col
    biasp[:, 2] = l2_b[0:4].reshape(128)
    biasp[:, 3] = l2_b[4:8].reshape(128)
    biasp[64:72, 4] = l1_b[:, L2] + out_b[:, 0]
    cw16 = np.concatenate([w2p, w3p], axis=1)  # [128, 544] f16
    cw32 = np.concatenate([wcp, biasp], axis=1).astype(f4)  # [128, 24] f32
    return {"w1t": w1t, "cw16": cw16, "cw32": cw32}


_cache = {}
_last_results = None


def kernel(x, router_w, router_b, l1_w, l1_b, l2_w, l2_b, out_w, out_b):
    global _last_results
    x = np.asarray(x, dtype=np.float32)
    weights = prep_weights(
        np.asarray(router_w, np.float32),
        np.asarray(router_b, np.float32),
        np.asarray(l1_w, np.float32),
        np.asarray(l1_b, np.float32),
        np.asarray(l2_w, np.float32),
        np.asarray(l2_b, np.float32),
        np.asarray(out_w, np.float32),
        np.asarray(out_b, np.float32),
    )

    xh = x.astype(np.float16)
    in_maps = []
    for core in range(N_CORES):
        shard = xh[core * B_SH : (core + 1) * B_SH]  # [2048, 3072] f16
        # xp[p, b, c, m] = shard[b*MB + m, c*128 + p]
        xp = np.ascontiguousarray(
            shard.reshape(NB, MB, KC, 128).transpose(3, 0, 2, 1)
        ).reshape(128, NB * KC * MB)
        sh32 = x[core * B_SH : (core + 1) * B_SH]
        xr = np.ascontiguousarray(
            np.concatenate([sh32[:, :RF], sh32[:, HALF : HALF + RF]], axis=1).T
        )  # [64, 2048] f32
        in_maps.append({"xp": xp, "xr": xr, **weights})

    if "nc" not in _cache:
        _cache["nc"] = build_nc()
    nc = _cache["nc"]

    from concourse.bass_utils import run_bass_kernel_spmd

    trace = bool(int(os.environ.get("KERNEL_TRACE", "0")))
    try:
        res = run_bass_kernel_spmd(
            nc, in_maps, core_ids=list(range(N_CORES)), trace=trace
        )
    except Exception:
        if not trace:
            raise
        res = run_bass_kernel_spmd(
            nc, in_maps, core_ids=list(range(N_CORES)), trace=False
        )
    _last_results = res
    # y[p, g] = out row g*128 + p within the core shard
    out = np.concatenate(
        [np.ascontiguousarray(r["y"].T).reshape(B_SH, 1) for r in res.results], axis=0
    )
    return out


# revision 50
# speedup vs baseline: 1.0717x; 1.0717x over previous
"""Trainium2 Bass kernel for nn_MoELayerStacks (moe_routing).

Full inputs in, full output out. Data-parallel over batch across 8 cores.

Math (per batch row b):
  gate = [x[:32], x[1536:1568]] @ router_w.T + router_b           # [8]
  idx  = argmax(gate)
  l1c  = x @ l1_w[e].T + l1_b[e]   for all e                      # [8, 16]
  l1x  = clip([square(l1c[:, :15])*255/256, l1c[:, :15]], 0, 1)   # [8, 30]
  l2x  = clip(l1x @ l2_w[e].T + l2_b[e], 0, 1)                    # [8, 32]
  out  = (l2x @ out_w[e].T + out_b[e] + l1c[:, 15])[idx]          # [1]

v2 design (vs the fp32r v1): x and the expert weights are cast to fp16 on
the host, halving HBM->SBUF traffic (the dominant cost) and keeping the PE
at 1 cyc/row. The 64 router features ride in a separate fp32 sidecar so the
argmax sees the exact fp32 gate dot products (identical numerics to v1).

Layouts: features on partitions, batch on the free dim for l1/l2. Stacked
l1 feature index r(e,o): l1x features (k = o*8+e) at r = k for k < 64 and
r = k+8 for k >= 64; the 8 l1x_out features at r = 64+e so a lane-aligned
DVE copy can drop them into rows 64..71 of the fp32 gate-stationary tile.

The gate, l3, and argmax-select run batch-major without any PE transposes:
per 128-column chunk j, one fp32 matmul with stationary xr_ext[:, j] (64 xr
rows + 8 l1x_out rows + a ones row for router_b) and moving wcomb [97, 16]
produces gate (cols 0..7) and l1x_out (cols 8..15, via an identity block);
two fp16 matmuls with stationary l2a/l2b chunks and moving w3e accumulate
the l3 contribution into cols 8..15. A short DVE chain (reduce_max, is_ge,
mult, reduce_sum) then emits the selected expert output per batch row.
"""

import os
from contextlib import ExitStack

import numpy as np

import concourse.bacc as bacc
import concourse.mybir as mybir
import concourse.tile as tile

N_CORES = 8
B, L1, L2, L3, E = 16384, 3072, 15, 32, 8
RF = 32  # router feats per perspective
HALF = L1 // 2
B_SH = B // N_CORES  # 2048 rows per core
KC = L1 // 128  # 24 contraction chunks
SQ_SCALE = float(np.sqrt(255.0 / 256.0))
MB = 256  # batch columns per block
NB = B_SH // MB  # 8 blocks
NSUB = MB // 128  # 2 128-col chunks per block

F32 = mybir.dt.float32
F16 = mybir.dt.float16
ALU = mybir.AluOpType


def _stack_row(k):
    """Stacked l1 partition for l1x feature k = o*8+e (l1x_out at 64..71)."""
    return k if k < 64 else k + 8


def build_nc():
    nc = bacc.Bacc(dynamic_dma_scratch_size=2048)

    xp = nc.dram_tensor("xp", [128, NB * KC * MB], F16, kind="ExternalInput")
    xr = nc.dram_tensor("xr", [2 * RF, B_SH], F32, kind="ExternalInput")
    w1t = nc.dram_tensor("w1t", [128, KC * 128], F16, kind="ExternalInput")
    cw16 = nc.dram_tensor("cw16", [128, 544], F16, kind="ExternalInput")
    cw32 = nc.dram_tensor("cw32", [128, 24], F32, kind="ExternalInput")
    y = nc.dram_tensor("y", [128, NB * NSUB], F32, kind="ExternalOutput")

    with tile.TileContext(nc) as tc, ExitStack() as ctx:
        const = ctx.enter_context(tc.tile_pool(name="const", bufs=1))
        xpool = ctx.enter_context(tc.tile_pool(name="x", bufs=6))
        actp = ctx.enter_context(tc.tile_pool(name="act", bufs=2))
        smallp = ctx.enter_context(tc.tile_pool(name="small", bufs=2))
        ps_1 = ctx.enter_context(tc.tile_pool(name="ps1", bufs=3, space="PSUM"))
        ps_2a = ctx.enter_context(tc.tile_pool(name="ps2a", bufs=1, space="PSUM"))
        ps_2b = ctx.enter_context(tc.tile_pool(name="ps2b", bufs=1, space="PSUM"))
        ps_sel = ctx.enter_context(tc.tile_pool(name="psel", bufs=2, space="PSUM"))

        w1t_sb = const.tile([128, KC, 128], F16)
        w1t_v = w1t[:, :].rearrange("p (c f) -> p c f", f=128)
        c16_sb = const.tile([128, 544], F16)
        c32_sb = const.tile([128, 24], F32)
        w2_sb = c16_sb[:, 0:512]
        w3a_sb = c16_sb[:, 512:528]
        w3b_sb = c16_sb[:, 528:544]
        wc_sb = c32_sb[:, 0:16]
        bias_sb = c32_sb[:, 16:24]
        xre = const.tile([128, B_SH], F32)  # rows 0..63 xr, 64..71 l1x_out, 96 ones
        yfull = const.tile([128, NB * NSUB], F32)

        def emit_consts():
            # everything the l1 matmuls and tails need, all on the sync queue
            # so x pieces on the other queues are never stuck behind them
            nc.sync.dma_start(c16_sb[:], cw16[:, :])
            nc.sync.dma_start(c32_sb[:], cw32[:, :])
            nc.scalar.dma_start(xre[0 : 2 * RF, :], xr[:, :])
            nc.vector.memset(xre[64:128, :], 0.0)
            nc.vector.memset(xre[96:97, :], 1.0)

        st = {}

        def emit_load(b, cuts=None, engs=None):
            xt = xpool.tile([128, KC, MB], F16, tag="xt")
            off = b * KC * MB
            cuts = cuts or [0, 6, 12, 18, 24]
            engs = engs or ([nc.sync, nc.scalar] * 8)
            src = xp[:, off : off + KC * MB].rearrange("p (c m) -> p c m", m=MB)
            for i in range(len(cuts) - 1):
                engs[i].dma_start(
                    xt[:, cuts[i] : cuts[i + 1], :], src[:, cuts[i] : cuts[i + 1], :]
                )
            st[b] = {"xt": xt}

        def emit_burst(b):
            xt = st[b]["xt"]
            ps1 = ps_1.tile([128, MB], F32, tag="ps1")
            for c in range(KC):
                nc.tensor.matmul(
                    ps1[:],
                    w1t_sb[:, c, :],
                    xt[:, c, :],
                    start=(c == 0),
                    stop=(c == KC - 1),
                )
            st[b]["ps1"] = ps1

        def emit_tail(b, nsplit=1, use_act=False):
            ps1 = st[b]["ps1"]
            psel = ps_sel.tile([128, NSUB, 16], F32, tag="psel")
            ps2a = ps_2a.tile([128, MB], F32, tag="ps2a")
            ps2b = ps_2b.tile([128, MB], F32, tag="ps2b")
            sq = actp.tile([128, MB], F16, tag="sq")
            raw = actp.tile([128, MB], F16, tag="raw")
            l2a = actp.tile([128, MB], F16, tag="l2a")
            l2b = actp.tile([128, MB], F16, tag="l2b")
            mx = smallp.tile([128, NSUB], F32, tag="mx")
            eq = smallp.tile([128, NSUB, 8], F32, tag="eq")
            prod = smallp.tile([128, NSUB, 8], F32, tag="prod")
            w = MB // nsplit
            for s in range(nsplit):
                cs = slice(s * w, (s + 1) * w)
                m0 = b * MB + s * w
                # l1x_out (+ l1 bias + out_b) into the fp32 gate-stationary rows
                if use_act:
                    nc.scalar.activation(
                        xre[64:72, m0 : m0 + w],
                        ps1[64:72, cs],
                        mybir.ActivationFunctionType.Identity,
                        bias=bias_sb[64:72, 4:5],
                    )
                else:
                    nc.vector.tensor_scalar(
                        xre[64:72, m0 : m0 + w],
                        ps1[64:72, cs],
                        bias_sb[64:72, 4:5],
                        None,
                        op0=ALU.add,
                    )
                # sq = min(1, (s*(l1c+b1))^2), raw = min(1, max(0, l1c+b1)).
                # PSUM-reading first steps on DVE/Act (GpSimd can't touch
                # PSUM); SBUF-only follow-ups on GpSimd — three parallel
                # elementwise tracks.
                nc.vector.tensor_scalar(
                    sq[:, cs],
                    ps1[:, cs],
                    bias_sb[:, 1:2],
                    SQ_SCALE,
                    op0=ALU.add,
                    op1=ALU.mult,
                )
                nc.vector.tensor_tensor(sq[:, cs], sq[:, cs], sq[:, cs], op=ALU.mult)
                nc.vector.tensor_scalar_min(sq[:, cs], sq[:, cs], 1.0)
                if use_act:
                    nc.scalar.activation(
                        raw[:, cs],
                        ps1[:, cs],
                        mybir.ActivationFunctionType.Relu,
                        bias=bias_sb[:, 1:2],
                    )
                else:
                    nc.vector.tensor_scalar(
                        raw[:, cs],
                        ps1[:, cs],
                        bias_sb[:, 1:2],
                        0.0,
                        op0=ALU.add,
                        op1=ALU.max,
                    )
                nc.vector.tensor_scalar_min(raw[:, cs], raw[:, cs], 1.0)

                # l2: two expert groups (0-3, 4-7), sq+raw accumulated
                nc.tensor.matmul(
                    ps2a[:, cs], w2_sb[:, 0:128], sq[:, cs], start=True, stop=False
                )
                nc.tensor.matmul(
                    ps2a[:, cs], w2_sb[:, 128:256], raw[:, cs], start=False, stop=True
                )
                nc.tensor.matmul(
                    ps2b[:, cs], w2_sb[:, 256:384], sq[:, cs], start=True, stop=False
                )
                nc.tensor.matmul(
                    ps2b[:, cs], w2_sb[:, 384:512], raw[:, cs], start=False, stop=True
                )

                if use_act:
                    nc.scalar.activation(
                        l2a[:, cs],
                        ps2a[:, cs],
                        mybir.ActivationFunctionType.Relu,
                        bias=bias_sb[:, 2:3],
                    )
                else:
                    nc.vector.tensor_scalar(
                        l2a[:, cs],
                        ps2a[:, cs],
                        bias_sb[:, 2:3],
                        0.0,
                        op0=ALU.add,
                        op1=ALU.max,
                    )
                nc.vector.tensor_scalar_min(l2a[:, cs], l2a[:, cs], 1.0)
                nc.vector.tensor_scalar(
                    l2b[:, cs], ps2b[:, cs], bias_sb[:, 3:4], 0.0, op0=ALU.add, op1=ALU.max
                )
                nc.vector.tensor_scalar_min(l2b[:, cs], l2b[:, cs], 1.0)

                # batch-major gate + all_outputs: per 128-col chunk j, PSUM
                # [128, 16]: cols 0..7 gate (fp32, exact), 8..15 l1x_out + l3c
                j0, j1 = s * (NSUB // nsplit), (s + 1) * (NSUB // nsplit)
                for j in range(j0, j1):
                    c0 = b * MB + j * 128
                    nc.tensor.matmul(
                        psel[:, j, :],
                        xre[0:97, c0 : c0 + 128],
                        wc_sb[0:97, :],
                        start=True,
                        stop=False,
                    )
                    nc.tensor.matmul(
                        psel[:, j, :],
                        l2a[:, j * 128 : (j + 1) * 128],
                        w3a_sb[:],
                        start=False,
                        stop=False,
                        skip_group_check=True,
                    )
                    nc.tensor.matmul(
                        psel[:, j, :],
                        l2b[:, j * 128 : (j + 1) * 128],
                        w3b_sb[:],
                        start=False,
                        stop=True,
                        skip_group_check=True,
                    )

                # argmax-select, batch on partitions
                js = slice(j0, j1)
                nc.vector.reduce_max(
                    mx[:, js], psel[:, js, 0:8], axis=mybir.AxisListType.X
                )
                for j in range(j0, j1):
                    nc.vector.tensor_scalar(
                        eq[:, j, :],
                        psel[:, j, 0:8],
                        mx[:, j : j + 1],
                        None,
                        op0=ALU.is_ge,
                    )
                nc.vector.tensor_tensor(
                    prod[:, js], eq[:, js], psel[:, js, 8:16], op=ALU.mult
                )
                ycols = slice(b * NSUB + j0, b * NSUB + j1)
                nc.vector.reduce_sum(
                    yfull[:, ycols], prod[:, js], axis=mybir.AxisListType.X
                )
            del st[b]

        # skewed software pipeline: PE runs burst(b) before tail(b-1) so the
        # DVE tail chain of block b-1 has a full block of slack. Two DMA
        # queues with disjoint roles: scalar carries only block 0 (fine
        # pieces for a fast PE start) and the router sidecar, then stays
        # empty; sync carries weights then blocks 1..3 in consumption order.
        # A single queue spreads its descriptors over all 16 DMA engines, so
        # bandwidth does not suffer. Tail elementwise work stays on the
        # vector engine except the last two blocks, where the scalar queue
        # has drained and the Act engine takes a parallel share.
        nc.sync.dma_start(w1t_sb[:, :, :], w1t_v[:, :, :])
        emit_load(
            0,
            cuts=[0, 3, 6, 12, 18, 24],
            engs=[nc.scalar, nc.sync, nc.scalar, nc.sync, nc.scalar],
        )
        emit_consts()
        emit_burst(0)
        for b in range(1, NB):
            if b == NB - 1:
                emit_load(
                    b,
                    cuts=[0, 12, 24],
                    engs=[nc.sync, nc.scalar],
                )
            else:
                emit_load(
                    b,
                    cuts=[0, 24],
                    engs=[nc.sync] if b % 2 else [nc.scalar],
                )
            emit_burst(b)
            emit_tail(b - 1, use_act=(b - 1 >= NB - 2))
        emit_tail(NB - 1, nsplit=2, use_act=True)
        nc.sync.dma_start(y[:, :], yfull[:])

    nc.finalize()
    return nc


def prep_weights(router_w, router_b, l1_w, l1_b, l2_w, l2_b, out_w, out_b):
    """Host-side packing of the (tiny) weights into the kernel's layouts."""
    f4, f2 = np.float32, np.float16
    # stacked l1 rows: l1x k=o*8+e -> r(k); l1x_out e -> 64+e
    w1_stacked = np.zeros((128, L1), f4)
    b1col = np.zeros(128, f4)
    for o in range(L2):
        for e in range(E):
            r = _stack_row(o * 8 + e)
            w1_stacked[r] = l1_w[e, o, :]
            b1col[r] = l1_b[e, o]
    for e in range(E):
        w1_stacked[64 + e] = l1_w[e, L2, :]
        b1col[64 + e] = l1_b[e, L2]
    w1t_kf = np.ascontiguousarray(w1_stacked.T).astype(f2)  # [L1, 128]
    # swizzle to [p, c, f] so the on-chip load is contiguous per partition
    w1t = np.ascontiguousarray(
        np.transpose(w1t_kf.reshape(KC, 128, 128), (1, 0, 2))
    ).reshape(128, KC * 128)
    # l2 weights: rows r(e,o), packed [sqA | rawA | sqB | rawB]
    w2p = np.zeros((128, 512), f4)
    for e in range(E):
        base = 0 if e < 4 else 256
        c0 = (e % 4) * 32
        wt = l2_w[e].T  # [30, 32]; rows 0..14 sq features, 15..29 raw
        rows = np.array([_stack_row(o * 8 + e) for o in range(L2)])
        w2p[rows, base + c0 : base + c0 + 32] = wt[0:L2]
        w2p[rows, base + 128 + c0 : base + 128 + c0 + 32] = wt[L2 : 2 * L2]
    w2p = w2p.astype(f2)
    # l3 (batch-major): w3p[:, g*16 + 8 + e] over the 32-feature band of e
    w3p = np.zeros((128, 32), f4)
    for e in range(E):
        g = e // 4
        w3p[(e % 4) * 32 : (e % 4) * 32 + 32, g * 16 + 8 + e] = out_w[e, 0, :]
    w3p = w3p.astype(f2)
    # wcomb: rows 0..63 router_w.T -> gate cols; rows 64..71 identity -> l1x_out
    # passthrough; row 96 (ones row in xre) carries router_b
    wcp = np.zeros((128, 16), f4)
    wcp[0 : 2 * RF, 0:8] = router_w.T
    for e in range(E):
        wcp[64 + e, 8 + e] = 1.0
    wcp[96, 0:8] = router_b
    biasp = np.zeros((128, 8), f4)
    biasp[:, 1] = b1col
    biasp[:, 2] = l2_b[0:4].reshape(128)
    biasp[:, 3] = l2_b[4:8].reshape(128)
    biasp[64:72, 4] = l1_b[:, L2] + out_b[:, 0]
    cw16 = np.concatenate([w2p, w3p], axis=1)  # [128, 544] f16
    cw32 = np.concatenate([wcp, biasp], axis=1).astype(f4)  # [128, 24] f32
    return {"w1t": w1t, "cw16": cw16, "cw32": cw32}


_cache = {}
_last_results = None


def kernel(x, router_w, router_b, l1_w, l1_b, l2_w, l2_b, out_w, out_b):
    global _last_results
    x = np.asarray(x, dtype=np.float32)
    weights = prep_weights(
        np.asarray(router_w, np.float32),
        np.asarray(router_b, np.float32),
        np.asarray(l1_w, np.float32),
        np.asarray(l1_b, np.float32),
        np.asarray(l2_w, np.float32),
        np.asarray(l2_b, np.float32),
        np.asarray(out_w, np.float32),
        np.asarray(out_b, np.float32),
    )

    xh = x.astype(np.float16)
    in_maps = []
    for core in range(N_CORES):
        shard = xh[core * B_SH : (core + 1) * B_SH]  # [2048, 3072] f16
        # xp[p, b, c, m] = shard[b*MB + m, c*128 + p]
        xp = np.ascontiguousarray(
            shard.reshape(NB, MB, KC, 128).transpose(3, 0, 2, 1)
        ).reshape(128, NB * KC * MB)
        sh32 = x[core * B_SH : (core + 1) * B_SH]
        xr = np.ascontiguousarray(
            np.concatenate([sh32[:, :RF], sh32[:, HALF : HALF + RF]], axis=1).T
        )  # [64, 2048] f32
        in_maps.append({"xp": xp, "xr": xr, **weights})

    if "nc" not in _cache:
        _cache["nc"] = build_nc()
    nc = _cache["nc"]

    from concourse.bass_utils import run_bass_kernel_spmd

    trace = bool(int(os.environ.get("KERNEL_TRACE", "0")))
    try:
        res = run_bass_kernel_spmd(
            nc, in_maps, core_ids=list(range(N_CORES)), trace=trace
        )
    except Exception:
        if not trace:
            raise
        res = run_bass_kernel_spmd(
            nc, in_maps, core_ids=list(range(N_CORES)), trace=False
        )
    _last_results = res
    # y[p, g] = out row g*128 + p within the core shard
    out = np.concatenate(
        [np.ascontiguousarray(r["y"].T).reshape(B_SH, 1) for r in res.results], axis=0
    )
    return out
